# revision 1
# baseline (speedup 1.0000x reference)
"""ARD-RBF kernel matrix on 8 TRN2 NeuronCores.

Math (reference):
    alpha = softmax(alpha_raw^2)            (D,)
    var   = variance_raw^2                  scalar
    sq_ij = sum_d alpha_d (x1_id - x2_jd)^2
    out   = var * exp(-0.5 * sq)            (N, M) f32

Device formulation (rows of x1 sharded 8 ways; per core):
    out_ij = exp( cross_ij - 0.5*ra_i + ln var ) * exp(-0.5*rb_j)
    cross  = x1 @ (alpha * x2)^T            fp16 matmul, f32 PSUM accum
    exp(-0.5*rb_j) is exp'd on host, broadcast on-chip once (k=1 matmul
    of ones x erb_row through PSUM, VectorE cast-copy to SBUF), then one
    VectorE bf16 multiply per group. The per-row term rides the Exp
    activation's per-partition bias.

Host does only O(N*D) prep: softmax(alpha), row norms ra/rb, transposes,
fp16 casts. All O(N*M*D) matmul + O(N*M) exp + 256MB output IO on device.

Pipeline per core: PE (8 fp16 matmuls/group into f32 PSUM) -> ScalarE Exp
(+per-partition bias, PSUM -> bf16 SBUF) -> VectorE bf16 multiply by the
broadcast exp(-0.5 rb) row -> 512KB bf16 output DMA chunks (host upcasts
to f32; adds <0.2% error, halves output bytes). The erb row is pre-exp'd
on host (fp16), broadcast on-chip by rank-1 matmuls, drained PSUM->SBUF
by VectorE cast-copies. 2 PSUM group buffers, 3 output tile buffers;
critical path is ScalarE's exp stream (~63us) + the ~12us input load.
"""

import math
import sys

import numpy as np

import ml_dtypes  # noqa: F401  (np bf16 dtype for device results)

if "/opt/trn_rl_repo" not in sys.path:
    sys.path.insert(0, "/opt/trn_rl_repo")

N, M, D = 8192, 8192, 256
NCORES = 8
NS = N // NCORES          # 1024 rows of x1 per core
P = 128                   # partitions
KT = D // P               # 2 k-tiles
JG = 2048                 # ScalarE activation group (4 PSUM banks)
NJ = 512                  # matmul moving free dim (1 PSUM bank)

_F16 = np.float16

_compiled = None

# tunables (read at _build/_prep time; bench scripts may override)
WARM_FILL = 6    # filler warmup MMs between erb0 and main0
TILE_H = 128     # output rows per i-tile; partial-partition DMA issue is ~10x slower, keep 128
OCH = 1          # ScalarE groups per output DMA chunk (1 -> 1MB chunks, 2 -> 2MB)


def _heights():
    hs = []
    r = NS
    while r > 0:
        h = min(TILE_H, r)
        hs.append(h)
        r -= h
    return hs


def _build():
    import concourse.bass as bass
    import concourse.mybir as mybir
    from contextlib import ExitStack

    dt = mybir.dt
    nc = bass.Bass()

    HS = _heights()
    NTL = len(HS)
    ROFF = [sum(HS[:i]) for i in range(NTL)]

    x1t = nc.declare_dram_parameter("x1t", [KT, P, NS], dt.float16, isOutput=False)
    x2t = nc.declare_dram_parameter("x2t", [KT, P, M], dt.float16, isOutput=False)
    rbn = nc.declare_dram_parameter("rbn", [1, M], dt.float16, isOutput=False)
    one = nc.declare_dram_parameter("one", [1, P], dt.float16, isOutput=False)
    bia = nc.declare_dram_parameter("bia", [P, NTL], dt.float32, isOutput=False)
    out = nc.declare_dram_parameter("out", [NS, M], dt.bfloat16, isOutput=True)

    ngi = M // JG        # 4 ScalarE groups per i-tile
    njc = JG // NJ       # 4 matmul column chunks per group
    PS_BUFS = 2
    OT_BUFS = 3
    NEG = ngi            # 4 erb pre-groups
    G_TOT = NTL * ngi    # main groups
    exp_f = mybir.ActivationFunctionType.Exp

    with ExitStack() as _ctx:
        ec = _ctx.enter_context
        x1s = ec(nc.sbuf_tensor("x1s", [P, KT * NS], dt.float16))
        x2s = ec(nc.sbuf_tensor("x2s", [P, KT * M], dt.float16))
        rbs = ec(nc.sbuf_tensor("rbs", [1, M], dt.float16))
        ons = ec(nc.sbuf_tensor("ons", [1, P], dt.float16))
        bis = ec(nc.sbuf_tensor("bis", [P, NTL], dt.float32))
        erb = ec(nc.sbuf_tensor("erb", [P, M], dt.bfloat16))
        ot0 = ec(nc.sbuf_tensor("ot0", [P, M], dt.bfloat16))
        ot1 = ec(nc.sbuf_tensor("ot1", [P, M], dt.bfloat16))
        ot2 = ec(nc.sbuf_tensor("ot2", [P, M], dt.bfloat16))
        wrm = ec(nc.sbuf_tensor("wrm", [P, P + NJ], dt.float16))  # uninit junk
        scr = ec(nc.sbuf_tensor("scr", [1, 32], dt.float32))      # table preload
        ps0 = ec(nc.psum_tensor("ps0", [P, JG], dt.float32))
        ps1 = ec(nc.psum_tensor("ps1", [P, JG], dt.float32))
        # DMA-completion sems: the 16 SDMA engines inc independently, so only
        # FULL-count waits on a sem are race-free -> one sem per group.
        s_ab = ec(nc.semaphore("s_ab"))    # rbs + ons      (full = 32)
        s_x1 = ec(nc.semaphore("s_x1"))    # x1 k0 + k1     (full = 32)
        s_x2a = ec(nc.semaphore("s_x2a"))  # x2 col-group 0 (full = 32)
        s_x2b = ec(nc.semaphore("s_x2b"))  # x2 col-group 1 (full = 32)
        s_x2c = ec(nc.semaphore("s_x2c"))  # x2 col-group 2 (full = 32)
        s_x2d = ec(nc.semaphore("s_x2d"))  # x2 col-group 3 (full = 32)
        s_bi = ec(nc.semaphore("s_bi"))    # bias           (full = 16)
        dp0 = ec(nc.semaphore("dp0"))      # out chunks, tiles t%3==0
        dp1 = ec(nc.semaphore("dp1"))      # out chunks, tiles t%3==1
        dp2 = ec(nc.semaphore("dp2"))      # out chunks, tiles t%3==2
        pes = ec(nc.semaphore("pes"))
        acs = ec(nc.semaphore("acs"))
        vcs = ec(nc.semaphore("vcs"))
        block = ec(nc.Block())
        ots = [ot0, ot1, ot2]
        pss = [ps0, ps1]
        dps_l = [dp0, dp1, dp2]
        s_x2_l = [s_x2a, s_x2b, s_x2c, s_x2d]

        # erb broadcast groups interleaved with the first main groups:
        # PSUM-use order = eg0, G0, eg1, G1, eg2, G2, eg3, G3, G4, ...
        USES = []
        for i in range(NEG):
            USES.append(("erb", i))
            USES.append(("main", i))
        for G in range(NEG, G_TOT):
            USES.append(("main", G))
        # ACT runs only main groups (acs tick = G order); DVE does one op per
        # psum use (erb -> cast-copy, main -> mul), so vcs tick = use index + 1.
        MAIN_ORD = [i for k, i in USES if k == "main"]
        ACS_MAIN = {G: n + 1 for n, G in enumerate(MAIN_ORD)}
        VCS_MAIN = {i: u + 1 for u, (k, i) in enumerate(USES) if k == "main"}

        @block.sync
        def _(sync):
            def x2_chunk(g):
                for k in range(KT):
                    sync.dma_start(
                        x2s[:, k * M + g * JG: k * M + (g + 1) * JG],
                        x2t[k, :, g * JG:(g + 1) * JG],
                    ).then_inc(s_x2_l[g], 16)

            sync.dma_start(rbs[:, :], rbn[:, :]).then_inc(s_ab, 16)
            sync.dma_start(ons[:, :], one[:, :]).then_inc(s_ab, 16)
            x2_chunk(0)                      # col-group 0 early: un-gates t=0
            for k in range(KT):
                sync.dma_start(x1s[:, k * NS:(k + 1) * NS], x1t[k]).then_inc(s_x1, 16)
            sync.dma_start(bis[:, :], bia[:, :]).then_inc(s_bi, 16)
            for g in range(1, ngi):
                x2_chunk(g)
            assert ngi % OCH == 0
            for G in range(OCH - 1, G_TOT, OCH):
                t, g = divmod(G, ngi)
                H, R0 = HS[t], ROFF[t]
                g0 = g - (OCH - 1)
                sync.wait_ge(vcs, VCS_MAIN[G])
                sync.dma_start(
                    out[R0:R0 + H, g0 * JG:(g + 1) * JG],
                    ots[t % OT_BUFS][0:H, g0 * JG:(g + 1) * JG],
                ).then_inc(dps_l[t % OT_BUFS], 16)
            for sidx in range(OT_BUFS):
                ntiles = len([t for t in range(NTL) if t % OT_BUFS == sidx])
                sync.wait_ge(dps_l[sidx], 16 * (ngi // OCH) * ntiles)

        @block.tensor
        def _(tensor):
            # PE warm-up on junk data while inputs stream in
            for _ in range(10):
                tensor.matmul(ps0[:, 0:NJ], wrm[:, 0:P], wrm[:, P:P + NJ],
                              start=True, stop=True)
            first = True
            for u, (kind, idx) in enumerate(USES):
                if u == 1:
                    # keep PE warm while waiting for the first x2 chunk; ps1
                    # bank 0 is safe: its next reader is main0's ACT, which
                    # only runs after main0's start=True overwrite.
                    for _ in range(WARM_FILL):
                        tensor.matmul(ps1[:, 0:NJ], wrm[:, 0:P], wrm[:, P:P + NJ],
                                      start=True, stop=True)
                if u >= PS_BUFS:
                    pk, pi = USES[u - PS_BUFS]
                    if pk == "erb":
                        tensor.wait_ge(vcs, u - PS_BUFS + 1)
                    else:
                        tensor.wait_ge(acs, ACS_MAIN[pi])
                ps = pss[u % PS_BUFS]
                if kind == "erb":
                    if idx == 0:
                        tensor.wait_ge(s_ab, 32)          # rbs + ons
                    for j in range(njc):
                        col = idx * JG + j * NJ
                        mm = tensor.matmul(ps[:, j * NJ:(j + 1) * NJ],
                                           ons[0:1, :], rbs[0:1, col:col + NJ],
                                           start=True, stop=True)
                else:
                    G = idx
                    t, g = divmod(G, ngi)
                    H, R0 = HS[t], ROFF[t]
                    if first:
                        tensor.wait_ge(s_x1, 32)          # x1 loaded
                        first = False
                    if t == 0:
                        tensor.wait_ge(s_x2_l[g], 32)     # x2 chunks, group g
                    for k in range(KT):
                        for j in range(njc):
                            col = g * JG + j * NJ
                            mm = tensor.matmul(
                                ps[0:H, j * NJ:(j + 1) * NJ],
                                x1s[:, k * NS + R0: k * NS + R0 + H],
                                x2s[:, k * M + col: k * M + col + NJ],
                                start=(k == 0),
                                stop=(k == KT - 1),
                            )
                mm.then_inc(pes)

        @block.scalar
        def _(scalar):
            # touch Exp early so ACT_TABLE_LOAD overlaps the input DMAs
            scalar.activation(scr[0:1, 16:32], scr[0:1, 0:16], exp_f)
            scalar.wait_ge(s_bi, 16)
            for u, (kind, idx) in enumerate(USES):
                if kind == "erb":
                    continue                  # erb psum is drained by VectorE
                G = idx
                t, g = divmod(G, ngi)
                H = HS[t]
                scalar.wait_ge(pes, u + 1)
                if g == 0 and t >= OT_BUFS:
                    # out-chunks of tiles t-3, t-6, ... (same slot) done
                    scalar.wait_ge(dps_l[t % OT_BUFS],
                                   16 * (ngi // OCH) * (t // OT_BUFS))
                scalar.activation(
                    ots[t % OT_BUFS][0:H, g * JG:(g + 1) * JG],
                    pss[u % PS_BUFS][0:H, :],
                    exp_f,
                    bias=bis[0:H, t:t + 1],
                    scale=1.0,
                ).then_inc(acs)

        @block.vector
        def _(vector):
            for u, (kind, idx) in enumerate(USES):
                if kind == "erb":
                    vector.wait_ge(pes, u + 1)
                    vector.tensor_copy(erb[:, idx * JG:(idx + 1) * JG],
                                       pss[u % PS_BUFS][:, :]).then_inc(vcs)
                else:
                    G = idx
                    t, g = divmod(G, ngi)
                    H = HS[t]
                    vector.wait_ge(acs, ACS_MAIN[G])
                    sl = slice(g * JG, (g + 1) * JG)
                    vector.tensor_mul(ots[t % OT_BUFS][0:H, sl],
                                      ots[t % OT_BUFS][0:H, sl],
                                      erb[0:H, sl]).then_inc(vcs)

    return nc


def _prep(x1, x2, alpha_raw, variance_raw):
    x1 = np.ascontiguousarray(np.asarray(x1, dtype=np.float32))
    x2 = np.ascontiguousarray(np.asarray(x2, dtype=np.float32))
    ar = np.asarray(alpha_raw, dtype=np.float64).reshape(-1)
    vr = np.asarray(variance_raw, dtype=np.float64).reshape(-1)

    a2 = ar * ar
    e = np.exp(a2 - a2.max())
    alpha = e / e.sum()                                   # (D,) f64
    var = float(vr[0]) ** 2
    if var > 0.0:
        logvar, post = math.log(var), None
    else:
        logvar, post = 0.0, var

    b = alpha[None, :] * x2.astype(np.float64)            # (M, D)
    x2tm = np.ascontiguousarray(b.T.reshape(KT, P, M).astype(_F16))
    x1tm = np.ascontiguousarray(x1.T.reshape(KT, P, N).astype(_F16))

    ra = (x1.astype(np.float64) ** 2) @ alpha             # (N,)
    rb = (x2.astype(np.float64) ** 2) @ alpha             # (M,)
    bia = (-0.5 * ra + logvar).astype(np.float32)         # (N,)
    rbn = np.ascontiguousarray(np.exp(-0.5 * rb).astype(_F16).reshape(1, M))
    ones = np.ones((1, P), dtype=_F16)

    HS = _heights()
    NTL = len(HS)
    ROFF = [sum(HS[:i]) for i in range(NTL)]

    in_maps = []
    for c in range(NCORES):
        bslice = bia[c * NS:(c + 1) * NS]
        bia2 = np.zeros((P, NTL), dtype=np.float32)
        for t in range(NTL):
            bia2[0:HS[t], t] = bslice[ROFF[t]:ROFF[t] + HS[t]]
        in_maps.append({
            "x1t": np.ascontiguousarray(x1tm[:, :, c * NS:(c + 1) * NS]),
            "x2t": x2tm,
            "rbn": rbn,
            "one": ones,
            "bia": np.ascontiguousarray(bia2),
        })
    return in_maps, post


def _run(in_maps, trace=False):
    global _compiled
    from concourse.bass_utils import run_bass_kernel_spmd

    if _compiled is None:
        _compiled = _build()
    return run_bass_kernel_spmd(
        _compiled, in_maps, core_ids=list(range(NCORES)), trace=trace
    )


def kernel(x1, x2, alpha_raw, variance_raw):
    in_maps, post = _prep(x1, x2, alpha_raw, variance_raw)
    res = _run(in_maps)
    full = np.concatenate(
        [np.asarray(res.results[c]["out"]).astype(np.float32) for c in range(NCORES)],
        axis=0)
    if post is not None:
        full = (full * post).astype(np.float32)
    return full



# revision 13
# speedup vs baseline: 1.0648x; 1.0648x over previous
"""ARD-RBF kernel matrix on 8 TRN2 NeuronCores.

Math (reference):
    alpha = softmax(alpha_raw^2)            (D,)
    var   = variance_raw^2                  scalar
    sq_ij = sum_d alpha_d (x1_id - x2_jd)^2
    out   = var * exp(-0.5 * sq)            (N, M) f32

Device formulation (rows of x1 sharded 8 ways; per core):
    out_ij = exp( cross_ij - 0.5*ra_i + ln var ) * exp(-0.5*rb_j)
    cross  = x1 @ (alpha * x2)^T            fp16 matmul, f32 PSUM accum

The pipeline is paced by ScalarE (ACT), the only engine that can do exp:
8.4M exps/core at 1 elem/lane/cycle @1.2GHz = ~61us with per-op overhead.
Everything else hides behind it:
  PE      54.6us at full clock (2.4GHz) -- has slack, but only at full
          clock; PE downclocks to 1.2GHz after stalls, so the schedule
          must never starve it (see group order below).
  DVE     bf16 column-scale by exp(-0.5 rb): ~39us.
  DMA     16MB out + 4.5MB in at ~358GB/s/core: ~57us aggregate.

Group order is COLUMN-major (g outer, t inner): the first 8 groups all
read x2 column-group 0, so only bia+x1+1MB of x2 gate the start of the
pipe (vs all 4.5MB with row-major order).  x2 column groups g1..g3
stream in behind.  exp(-0.5 rb) is DMA'd as one 16KB row and replicated
to 128 partitions by GpSimd partition_broadcast (2048 cols per chunk),
off every critical path.

Per group G = g*8 + t: PE 8 fp16 matmuls (2 k-tiles x 4 x 512 cols) into
a [128,2048] f32 PSUM half; ACT Exp w/ per-partition bias (-0.5 ra + ln
var) -> bf16 ot[G%4]; DVE multiplies by erb[g] slice; sync issues the
512KB output chunk.  2 PSUM halves, 4 ot slots.  The last group's
mul+DMA are split in two to shorten the tail.
"""

import math
import sys

import numpy as np

import ml_dtypes  # noqa: F401  (np bf16 dtype for device results)

if "/opt/trn_rl_repo" not in sys.path:
    sys.path.insert(0, "/opt/trn_rl_repo")

N, M, D = 8192, 8192, 256
NCORES = 8
NS = N // NCORES          # 1024 rows of x1 per core
P = 128                   # partitions
KT = D // P               # 2 k-tiles
NG = 4                    # x2 column groups
JG = M // NG              # 2048 cols per group
NJ = 512                  # matmul moving free dim (1 PSUM bank)
NT = NS // P              # 8 row tiles per core
NGRP = NG * NT            # 32 groups

_F16 = np.float16
_BF16 = ml_dtypes.bfloat16

_compiled = None

# tunables
WARM = 10                 # junk warmup matmuls to ramp PE p-state


def _build():
    import concourse.bass as bass
    import concourse.mybir as mybir
    from contextlib import ExitStack

    dt = mybir.dt
    nc = bass.Bass()

    x1d = nc.declare_dram_parameter("x1d", [P, KT * NS], dt.float16, isOutput=False)
    x2d = nc.declare_dram_parameter("x2d", [P, NG * KT * JG], dt.float16, isOutput=False)
    rbd = nc.declare_dram_parameter("rbd", [P, M], dt.bfloat16, isOutput=False)
    biad = nc.declare_dram_parameter("biad", [P, NT], dt.float32, isOutput=False)
    outd = nc.declare_dram_parameter("out", [NS, M], dt.bfloat16, isOutput=True)

    exp_f = mybir.ActivationFunctionType.Exp
    njc = JG // NJ            # 4 matmul column chunks per group

    with ExitStack() as _ctx:
        ec = _ctx.enter_context
        x1s = ec(nc.sbuf_tensor("x1s", [P, KT * NS], dt.float16))
        x2s = ec(nc.sbuf_tensor("x2s", [P, NG * KT * JG], dt.float16))
        bis = ec(nc.sbuf_tensor("bis", [P, NT], dt.float32))
        erb = ec(nc.sbuf_tensor("erb", [P, M], dt.bfloat16))
        ot0 = ec(nc.sbuf_tensor("ot0", [P, JG], dt.bfloat16))
        ot1 = ec(nc.sbuf_tensor("ot1", [P, JG], dt.bfloat16))
        ot2 = ec(nc.sbuf_tensor("ot2", [P, JG], dt.bfloat16))
        ot3 = ec(nc.sbuf_tensor("ot3", [P, JG], dt.bfloat16))
        wrm = ec(nc.sbuf_tensor("wrm", [P, P + NJ], dt.float16))  # uninit junk
        scr = ec(nc.sbuf_tensor("scr", [1, 32], dt.float32))      # table preload
        ps0 = ec(nc.psum_tensor("ps0", [P, JG], dt.float32))
        ps1 = ec(nc.psum_tensor("ps1", [P, JG], dt.float32))
        # DMA-completion sems: the 16 SDMA engines inc independently, so only
        # FULL-count waits on a sem are race-free.
        s_in = ec(nc.semaphore("s_in"))    # bia + x1           (full = 32)
        s_x2a = ec(nc.semaphore("s_x2a"))  # x2 g0 k0           (full = 16)
        s_x2b = ec(nc.semaphore("s_x2b"))  # x2 g0 k1           (full = 16)
        s_x2c = ec(nc.semaphore("s_x2c"))  # x2 g1              (full = 16)
        s_x2d = ec(nc.semaphore("s_x2d"))  # x2 g2              (full = 16)
        s_x2e = ec(nc.semaphore("s_x2e"))  # x2 g3              (full = 16)
        eb0 = ec(nc.semaphore("eb0"))      # erb chunk per col group (full = 16)
        eb1 = ec(nc.semaphore("eb1"))
        eb2 = ec(nc.semaphore("eb2"))
        eb3 = ec(nc.semaphore("eb3"))
        dp0 = ec(nc.semaphore("dp0"))      # out chunks, slot 0
        dp1 = ec(nc.semaphore("dp1"))
        dp2 = ec(nc.semaphore("dp2"))
        dp3 = ec(nc.semaphore("dp3"))
        pes = ec(nc.semaphore("pes"))
        acs = ec(nc.semaphore("acs"))
        vcs = ec(nc.semaphore("vcs"))
        block = ec(nc.Block())
        ots = [ot0, ot1, ot2, ot3]
        pss = [ps0, ps1]
        dps = [dp0, dp1, dp2, dp3]
        ebs_l = [eb0, eb1, eb2, eb3]

        def gt(G):
            g, t = divmod(G, NT)
            return g, t

        @block.sync
        def _(sync):
            # output chunks only; inputs are issued from gpsimd/scalar queues
            for G in range(NGRP):
                g, t = gt(G)
                if G < NGRP - 1:
                    sync.wait_ge(vcs, G + 1)
                    sync.dma_start(
                        outd[t * P:(t + 1) * P, g * JG:(g + 1) * JG],
                        ots[G % 4][:, :],
                    ).then_inc(dps[G % 4], 16)
                else:
                    # split tail: issue each half as soon as its mul lands
                    h = JG // 2
                    sync.wait_ge(vcs, NGRP)
                    sync.dma_start(
                        outd[t * P:(t + 1) * P, g * JG:g * JG + h],
                        ots[G % 4][:, 0:h],
                    ).then_inc(dps[G % 4], 16)
                    sync.wait_ge(vcs, NGRP + 1)
                    sync.dma_start(
                        outd[t * P:(t + 1) * P, g * JG + h:(g + 1) * JG],
                        ots[G % 4][:, h:JG],
                    ).then_inc(dps[G % 4], 16)
            for s in range(4):
                uses = len([G for G in range(NGRP) if G % 4 == s])
                extra = 16 if s == (NGRP - 1) % 4 else 0
                sync.wait_ge(dps[s], 16 * uses + extra)

        @block.gpsimd
        def _(gpsimd):
            # single input queue, strict priority order: items land in the
            # order the pipeline first needs them.  erb chunk g lands just
            # before DVE's first column-group-g multiply.
            gpsimd.dma_start(bis[:, :], biad[:, :]).then_inc(s_in, 16)
            gpsimd.dma_start(x1s[:, :], x1d[:, :]).then_inc(s_in, 16)
            gpsimd.dma_start(x2s[:, 0:JG], x2d[:, 0:JG]).then_inc(s_x2a, 16)
            gpsimd.dma_start(x2s[:, JG:2 * JG], x2d[:, JG:2 * JG]).then_inc(s_x2b, 16)
            gpsimd.dma_start(erb[:, 0:JG], rbd[:, 0:JG]).then_inc(eb0, 16)
            gpsimd.dma_start(erb[:, JG:2 * JG], rbd[:, JG:2 * JG]).then_inc(eb1, 16)
            gpsimd.dma_start(x2s[:, 2 * JG:4 * JG], x2d[:, 2 * JG:4 * JG]).then_inc(s_x2c, 16)
            gpsimd.dma_start(erb[:, 2 * JG:3 * JG], rbd[:, 2 * JG:3 * JG]).then_inc(eb2, 16)
            gpsimd.dma_start(erb[:, 3 * JG:4 * JG], rbd[:, 3 * JG:4 * JG]).then_inc(eb3, 16)
            gpsimd.dma_start(x2s[:, 4 * JG:6 * JG], x2d[:, 4 * JG:6 * JG]).then_inc(s_x2d, 16)
            gpsimd.dma_start(x2s[:, 6 * JG:8 * JG], x2d[:, 6 * JG:8 * JG]).then_inc(s_x2e, 16)

        @block.tensor
        def _(tensor):
            # PE warm-up on junk data while inputs stream in (p-state ramp)
            for _ in range(WARM):
                tensor.matmul(ps0[:, 0:NJ], wrm[:, 0:P], wrm[:, P:P + NJ],
                              start=True, stop=True)
            for G in range(NGRP):
                g, t = gt(G)
                if G == 0:
                    tensor.wait_ge(s_in, 32)     # x1 (+bia) loaded
                    tensor.wait_ge(s_x2a, 16)    # x2 g0 k0
                if G == NT:
                    tensor.wait_ge(s_x2c, 16)    # entering column group 1
                if G == 2 * NT:
                    tensor.wait_ge(s_x2d, 16)
                if G == 3 * NT:
                    tensor.wait_ge(s_x2e, 16)
                if G >= 2:
                    tensor.wait_ge(acs, G - 1)   # psum half free
                ps = pss[G % 2]
                for k in range(KT):
                    if G == 0 and k == 1:
                        tensor.wait_ge(s_x2b, 16)
                    for j in range(njc):
                        mm = tensor.matmul(
                            ps[:, j * NJ:(j + 1) * NJ],
                            x1s[:, k * NS + t * P: k * NS + (t + 1) * P],
                            x2s[:, (g * KT + k) * JG + j * NJ:
                                   (g * KT + k) * JG + (j + 1) * NJ],
                            start=(k == 0),
                            stop=(k == KT - 1),
                        )
                mm.then_inc(pes)

        @block.scalar
        def _(scalar):
            # touch Exp early so ACT_TABLE_LOAD overlaps the input DMAs
            scalar.activation(scr[0:1, 16:32], scr[0:1, 0:16], exp_f)
            scalar.wait_ge(s_in, 32)
            for G in range(NGRP):
                g, t = gt(G)
                scalar.wait_ge(pes, G + 1)
                if G >= 4:
                    scalar.wait_ge(dps[G % 4], 16 * (G // 4))  # ot slot free
                scalar.activation(
                    ots[G % 4][:, :],
                    pss[G % 2][:, :],
                    exp_f,
                    bias=bis[:, t:t + 1],
                    scale=1.0,
                ).then_inc(acs)

        @block.vector
        def _(vector):
            for G in range(NGRP):
                g, t = gt(G)
                vector.wait_ge(acs, G + 1)
                if t == 0:
                    vector.wait_ge(ebs_l[g], 16)
                sl = slice(g * JG, (g + 1) * JG)
                if G < NGRP - 1:
                    vector.tensor_mul(ots[G % 4][:, :], ots[G % 4][:, :],
                                      erb[:, sl]).then_inc(vcs)
                else:
                    h = JG // 2
                    vector.tensor_mul(ots[G % 4][:, 0:h], ots[G % 4][:, 0:h],
                                      erb[:, g * JG:g * JG + h]).then_inc(vcs)
                    vector.tensor_mul(ots[G % 4][:, h:JG], ots[G % 4][:, h:JG],
                                      erb[:, g * JG + h:(g + 1) * JG]).then_inc(vcs)

    return nc


def _prep(x1, x2, alpha_raw, variance_raw):
    x1 = np.ascontiguousarray(np.asarray(x1, dtype=np.float32))
    x2 = np.ascontiguousarray(np.asarray(x2, dtype=np.float32))
    ar = np.asarray(alpha_raw, dtype=np.float64).reshape(-1)
    vr = np.asarray(variance_raw, dtype=np.float64).reshape(-1)

    a2 = ar * ar
    e = np.exp(a2 - a2.max())
    alpha = e / e.sum()                                   # (D,) f64
    var = float(vr[0]) ** 2
    if var > 0.0:
        logvar, post = math.log(var), None
    else:
        logvar, post = 0.0, var

    b = alpha[None, :] * x2.astype(np.float64)            # (M, D)
    x2tm = b.T.reshape(KT, P, M).astype(_F16)             # [k, p, col]
    # device layout: col index = g*(KT*JG) + k*JG + j
    x2c = np.ascontiguousarray(
        x2tm.reshape(KT, P, NG, JG).transpose(1, 2, 0, 3).reshape(P, NG * KT * JG))
    x1tm = x1.T.reshape(KT, P, N).astype(_F16)            # [k, p, row]

    ra = (x1.astype(np.float64) ** 2) @ alpha             # (N,)
    rb = (x2.astype(np.float64) ** 2) @ alpha             # (M,)
    bia = (-0.5 * ra + logvar).astype(np.float32)         # (N,)
    rbrow = np.exp(-0.5 * rb).astype(_BF16).reshape(1, M)
    rbd = np.ascontiguousarray(np.broadcast_to(rbrow, (P, M)))

    in_maps = []
    for c in range(NCORES):
        sl = slice(c * NS, (c + 1) * NS)
        x1c = np.ascontiguousarray(
            np.concatenate([x1tm[0][:, sl], x1tm[1][:, sl]], axis=1))
        bia2 = np.ascontiguousarray(
            bia[sl].reshape(NT, P).T.astype(np.float32))   # [p, t]
        in_maps.append({
            "x1d": x1c,
            "x2d": x2c,
            "rbd": rbd,
            "biad": bia2,
        })
    return in_maps, post


def _run(in_maps, trace=False):
    global _compiled
    from concourse.bass_utils import run_bass_kernel_spmd

    if _compiled is None:
        _compiled = _build()
    return run_bass_kernel_spmd(
        _compiled, in_maps, core_ids=list(range(NCORES)), trace=trace
    )


def kernel(x1, x2, alpha_raw, variance_raw):
    in_maps, post = _prep(x1, x2, alpha_raw, variance_raw)
    res = _run(in_maps)
    full = np.concatenate(
        [np.asarray(res.results[c]["out"]).astype(np.float32) for c in range(NCORES)],
        axis=0)
    if post is not None:
        full = (full * post).astype(np.float32)
    return full
